# revision 15
# baseline (speedup 1.0000x reference)
"""Trainium2 Bass kernel for nn_Column (nms_detection).

Computation (matches the reference exactly):
  out[t,k]  = sum_chw rec_field[t,chw] * weight[k,chw]        (32x512 <- contract 100000)
  pot       = out * (out > 10) ; spike = (out > 10)
  nspk[k]   = sum_t spike ; first[k] = min(32 - nspk, 31)
  values[k] = pot[first[k], k] ; v = max_k(values * (nspk>0)) * 32
  total     = nspk*values + nspk*v
  coef      = top-16 nonzero mask of total (== sequential argmax-suppress set)
  result    = spike * coef[broadcast]                          (32x512 of 0.0/1.0)

Distribution: contraction dim (100000) sharded 8 ways (12500 rows/core, padded
to 12544 = 98*128).  Inputs are cast to fp16 on the host; each core computes a
partial (32,512) fp32 psum with 98 accumulating PE matmuls (stationary X chunk
(128,32), moving W chunk (128,512), 1 cycle/row at fp16).  Partials are
combined with one 64KB fp32 AllReduce (a tiny 4B AllReduce issued at kernel
start acts as a barrier that absorbs launch skew).  Every core then redundantly
computes the tiny k-WTA epilogue; core 0's output is returned.
"""

import numpy as np

import concourse.bacc as bacc
import concourse.mybir as mybir
from concourse.tile import TileContext
from concourse.bass_utils import run_bass_kernel_spmd

T = 32               # timesteps
K = 512              # out_channels / features
CTOT = 100000        # in_channels * rf_size * length (1*50*2000)
NCORES = 8
SH = CTOT // NCORES  # 12500 contraction rows per core
NCH = 98             # 128-row contraction chunks per core
SHP = NCH * 128      # 12544 (zero padded)
GROUP = 7            # chunks per W DMA group  (7*512*128*2B = 896 KiB)
NG = NCH // GROUP    # 14 groups
THRESH = 10.0
F32 = mybir.dt.float32
F16 = mybir.dt.float16
BF16 = mybir.dt.bfloat16

_CACHE = {}


def _build_nc():
    nc = bacc.Bacc("TRN2", target_bir_lowering=False, debug=False, num_devices=NCORES)

    x_d = nc.dram_tensor("x", [128, NCH * T], F16, kind="ExternalInput")
    w_d = nc.dram_tensor("w", [128, NCH * K], F16, kind="ExternalInput")
    oc_d = nc.dram_tensor("onescol", [T, 1], F32, kind="ExternalInput")
    o32_d = nc.dram_tensor("ones32", [T, T], BF16, kind="ExternalInput")
    or_d = nc.dram_tensor("onesrow", [1, T], BF16, kind="ExternalInput")
    tp_d = nc.dram_tensor("tpos32", [T, 1], F32, kind="ExternalInput")
    z_d = nc.dram_tensor("zero1", [1, 1], F32, kind="ExternalInput")
    out_d = nc.dram_tensor("out", [T, K], F32, kind="ExternalOutput")

    with TileContext(nc) as tc:
        with (
            tc.tile_pool(name="sb", bufs=1) as sb,
            tc.tile_pool(name="wp", bufs=4) as wp,
            tc.tile_pool(name="ps", bufs=1, space="PSUM") as ps,
            tc.tile_pool(name="dram", bufs=1, space="DRAM") as dr,
        ):
            # ---- barrier AllReduce (4 bytes): aligns the 8 cores right at
            # kernel start so the big collectives later don't eat launch skew.
            zb = sb.tile([1, 1], F32)
            nc.gpsimd.memset(zb[:], 0.0)
            bzin = dr.tile([1, 1], F32)
            bzout = dr.tile([1, 1], F32, addr_space="Shared")
            nc.gpsimd.dma_start(bzin[:], zb[:])
            nc.gpsimd.collective_compute(
                "AllReduce",
                mybir.AluOpType.add,
                replica_groups=[list(range(NCORES))],
                ins=[bzin.opt()],
                outs=[bzout.opt()],
            )
            zsum = sb.tile([1, 1], F32)
            nc.scalar.dma_start(zsum[:], bzout[:])

            # X (784 KB) on the scalar HWDGE ring, W groups alternate between
            # the sync and scalar rings so both HWDGE FIFOs stream in parallel.
            # X lands in two pieces so the first matmul only waits for the
            # first GROUP's worth of X columns.
            xsb = sb.tile([128, NCH * T], F16)
            nc.scalar.dma_start(xsb[:, :GROUP * T], x_d[:, :GROUP * T])
            nc.scalar.dma_start(xsb[:, GROUP * T:], x_d[:, GROUP * T:])
            oc = sb.tile([T, 1], F32)
            nc.scalar.dma_start(oc[:], oc_d[:])
            o32 = sb.tile([T, T], BF16)
            nc.scalar.dma_start(o32[:], o32_d[:])
            orr = sb.tile([1, T], BF16)
            nc.scalar.dma_start(orr[:], or_d[:])
            tpos = sb.tile([T, 1], F32)
            nc.scalar.dma_start(tpos[:], tp_d[:])

            # ---- matmul: 98 accumulating X-stationary fp16 matmuls (N=512) --
            accum = ps.tile([T, K], F32)
            for g in range(NG):
                wt = wp.tile([128, GROUP * K], F16, tag="wt")
                eng = nc.sync if (g % 2 == 0) else nc.scalar
                eng.dma_start(wt[:], w_d[:, g * GROUP * K:(g + 1) * GROUP * K])
                for c in range(GROUP):
                    cc = g * GROUP + c
                    nc.tensor.matmul(
                        accum[:],
                        xsb[:, cc * T:(cc + 1) * T],
                        wt[:, c * K:(c + 1) * K],
                        start=(cc == 0),
                        stop=(cc == NCH - 1),
                    )

            # ---- reduce the (32,512) fp32 partial across the 8 cores.
            # 3-stage pairwise-AllReduce butterfly (recursive doubling): the
            # runtime's own mesh/RS+AG plans at this size are step-latency
            # bound (~14 ncfw steps); the butterfly needs only 3 stages of
            # one-peer exchanges.
            part = sb.tile([T, K], F32)
            nc.vector.tensor_copy(part[:], accum[:])
            bin_ = dr.tile([T, K], F32)
            nc.sync.dma_start(bin_[:], part[:])
            stages = [
                [[0, 1], [2, 3], [4, 5], [6, 7]],
                [[0, 2], [1, 3], [4, 6], [5, 7]],
                [[0, 4], [1, 5], [2, 6], [3, 7]],
            ]
            cur = bin_
            for gi, groups in enumerate(stages):
                nxt = dr.tile([T, K], F32, tag=f"bfly{gi}")
                nc.gpsimd.collective_compute(
                    "AllReduce",
                    mybir.AluOpType.add,
                    replica_groups=groups,
                    ins=[cur.opt()],
                    outs=[nxt.opt()],
                )
                cur = nxt
            ofull = sb.tile([T, K], F32)
            nc.sync.dma_start(ofull[:], cur[:])

            # ---- threshold fire ----
            # spike is 0.0/1.0 -> exact in bf16; lets the nspk matmul run at
            # 1 cycle/row.
            spike = sb.tile([T, K], BF16)
            nc.vector.tensor_scalar(spike[:], ofull[:], THRESH, None,
                                    op0=mybir.AluOpType.is_gt)
            pot = sb.tile([T, K], F32)
            nc.vector.scalar_tensor_tensor(pot[:], ofull[:], THRESH, ofull[:],
                                           op0=mybir.AluOpType.is_gt,
                                           op1=mybir.AluOpType.mult)

            # nspk broadcast to all 32 rows in one bf16 matmul (counts <= 32
            # are exact): ones(32,32).T @ spike
            nspkb_ps = ps.tile([T, K], F32)
            nc.tensor.matmul(nspkb_ps[:], o32[:], spike[:], start=True, stop=True)
            nrow = sb.tile([1, K], F32)
            nc.vector.tensor_copy(nrow[:], nspkb_ps[0:1, :])

            # onehot(t == clip(32-nspk,0,31)) == (nspk == 32 - t) except the
            # nspk==0 row-31 case, where pot is all-zero anyway.
            # values[k] = sum_t pot * onehot  (kept in fp32 for exactness)
            pv = sb.tile([T, K], F32)
            nc.vector.scalar_tensor_tensor(pv[:], nspkb_ps[:], tpos[:], pot[:],
                                           op0=mybir.AluOpType.is_equal,
                                           op1=mybir.AluOpType.mult)
            vals_ps = ps.tile([1, K], F32)
            nc.tensor.matmul(vals_ps[:], oc[:], pv[:], start=True, stop=True)

            # v = max(values * (nspk > 0)) * 32
            vm = sb.tile([1, K], F32)
            nc.vector.scalar_tensor_tensor(vm[:], nrow[:], 0.0, vals_ps[:],
                                           op0=mybir.AluOpType.is_gt,
                                           op1=mybir.AluOpType.mult)
            vmax = sb.tile([1, 1], F32)
            nc.vector.tensor_reduce(vmax[:], vm[:], axis=mybir.AxisListType.X,
                                    op=mybir.AluOpType.max)
            # vmax32 = vmax * 32 + 0.0 (the 0.0 comes from the barrier
            # AllReduce output, keeping it alive without numeric effect)
            vmax32 = sb.tile([1, 1], F32)
            nc.vector.scalar_tensor_tensor(vmax32[:], vmax[:], float(T), zsum[:],
                                           op0=mybir.AluOpType.mult,
                                           op1=mybir.AluOpType.add)

            # total = (values + vmax32) * nspk
            total = sb.tile([1, K], F32)
            nc.vector.scalar_tensor_tensor(total[:], vals_ps[:], vmax32[:],
                                           nrow[:],
                                           op0=mybir.AluOpType.add,
                                           op1=mybir.AluOpType.mult)

            # top-16 nonzero mask: two rounds of (8-max, match-replace-with-0).
            # Zero entries "win" as no-ops and never enter the mask, matching
            # the reference's invalid-winner (-1) behavior.
            work = sb.tile([1, K], F32)
            s8a = sb.tile([1, 8], F32)
            nc.vector.max(s8a[:], total[:])
            nc.vector.match_replace(work[:], s8a[:], total[:], 0.0)
            s8b = sb.tile([1, 8], F32)
            nc.vector.max(s8b[:], work[:])
            nc.vector.match_replace(work[:], s8b[:], work[:], 0.0)

            # coef: winner totals (~2600..3200), 0 elsewhere.  Only its sign
            # matters downstream, so bf16 is exact enough -> 1 cycle/row
            # broadcast matmul.
            coef = sb.tile([1, K], BF16)
            nc.vector.tensor_tensor(coef[:], total[:], work[:],
                                    mybir.AluOpType.subtract)

            # result = spike * (coef_broadcast > 0)
            coefb_ps = ps.tile([T, K], F32)
            nc.tensor.matmul(coefb_ps[:], orr[:], coef[:], start=True, stop=True)
            res = sb.tile([T, K], F32)
            nc.vector.scalar_tensor_tensor(res[:], coefb_ps[:], 0.0, spike[:],
                                           op0=mybir.AluOpType.is_gt,
                                           op1=mybir.AluOpType.mult)
            nc.sync.dma_start(out_d[:], res[:])

    nc.compile()
    return nc


def _get_nc():
    if "nc" not in _CACHE:
        _CACHE["nc"] = _build_nc()
    return _CACHE["nc"]


def _pack_inputs(rec_field, weight):
    X = np.asarray(rec_field, dtype=np.float32).reshape(T, CTOT).astype(np.float16)
    W = np.asarray(weight, dtype=np.float32).reshape(K, CTOT).astype(np.float16)
    oc = np.ones((T, 1), np.float32)
    o32 = np.ones((T, T), np.float32).astype(mybir_np_bf16())
    orr = np.ones((1, T), np.float32).astype(mybir_np_bf16())
    tp = (float(T) - np.arange(T, dtype=np.float32)).reshape(T, 1)
    z1 = np.zeros((1, 1), np.float32)
    in_maps = []
    for i in range(NCORES):
        xp = np.zeros((T, SHP), np.float16)
        xp[:, :SH] = X[:, i * SH:(i + 1) * SH]
        wp = np.zeros((K, SHP), np.float16)
        wp[:, :SH] = W[:, i * SH:(i + 1) * SH]
        # (contract, n) -> chunks (NCH,128,n) -> partition-major (128, NCH*n)
        xpk = np.ascontiguousarray(
            xp.T.reshape(NCH, 128, T).transpose(1, 0, 2).reshape(128, NCH * T))
        wpk = np.ascontiguousarray(
            wp.T.reshape(NCH, 128, K).transpose(1, 0, 2).reshape(128, NCH * K))
        in_maps.append({"x": xpk, "w": wpk, "onescol": oc, "ones32": o32,
                        "onesrow": orr, "tpos32": tp, "zero1": z1})
    return in_maps


def mybir_np_bf16():
    import ml_dtypes
    return ml_dtypes.bfloat16


def kernel(rec_field, weight, _trace=False, _trace_kwargs=None):
    nc = _get_nc()
    in_maps = _pack_inputs(rec_field, weight)
    r = run_bass_kernel_spmd(nc, in_maps, list(range(NCORES)), trace=_trace,
                             **(_trace_kwargs or {}))
    _CACHE["last_results"] = r
    out = np.asarray(r.results[0]["out"], dtype=np.float32)
    return out.reshape(T, K, 1, 1)


# revision 24
# speedup vs baseline: 1.1997x; 1.1997x over previous
"""Trainium2 Bass kernel for nn_Column (nms_detection).

Computation (matches the reference exactly):
  out[t,k]  = sum_chw rec_field[t,chw] * weight[k,chw]        (32x512 <- contract 100000)
  pot       = out * (out > 10) ; spike = (out > 10)
  nspk[k]   = sum_t spike ; first[k] = min(32 - nspk, 31)
  values[k] = pot[first[k], k] ; v = max_k(values * (nspk>0)) * 32
  total     = nspk*values + nspk*v
  coef      = top-16 nonzero mask of total (== sequential argmax-suppress set)
  result    = spike * coef[broadcast]                          (32x512 of 0.0/1.0)

Distribution: contraction dim (100000) sharded 8 ways (12500 rows/core, padded
to 12544 = 98*128).  Inputs are cast to fp16 on the host; each core computes a
partial (32,512) fp32 psum with 98 accumulating PE matmuls (stationary X chunk
(128,32), moving W chunk (128,512), 1 cycle/row at fp16).  Partials are
combined with one 64KB fp32 AllReduce (a tiny 4B AllReduce issued at kernel
start acts as a barrier that absorbs launch skew).  Every core then redundantly
computes the tiny k-WTA epilogue; core 0's output is returned.
"""

import numpy as np

import concourse.bacc as bacc
import concourse.mybir as mybir
from concourse.tile import TileContext
from concourse.bass_utils import run_bass_kernel_spmd

T = 32               # timesteps
K = 512              # out_channels / features
CTOT = 100000        # in_channels * rf_size * length (1*50*2000)
NCORES = 8
SH = CTOT // NCORES  # 12500 contraction rows per core
NCH = 98             # 128-row contraction chunks per core
SHP = NCH * 128      # 12544 (zero padded)
GROUP = 14           # steady-state chunks per W DMA group (1.75 MiB)
GROUPS = [2, 5, 7] + [14] * 6   # per-group chunk counts (sums to 98)
THRESH = 10.0
F32 = mybir.dt.float32
F16 = mybir.dt.float16
BF16 = mybir.dt.bfloat16

_CACHE = {}


def _build_nc():
    nc = bacc.Bacc("TRN2", target_bir_lowering=False, debug=False, num_devices=NCORES)

    x_d = nc.dram_tensor("x", [128, NCH * T], F16, kind="ExternalInput")
    w_d = nc.dram_tensor("w", [128, NCH * K], F16, kind="ExternalInput")
    oc_d = nc.dram_tensor("onescol", [T, 1], F32, kind="ExternalInput")
    o32_d = nc.dram_tensor("ones32", [T, T], BF16, kind="ExternalInput")
    or_d = nc.dram_tensor("onesrow", [1, T], BF16, kind="ExternalInput")
    tp_d = nc.dram_tensor("tpos32", [T, 1], F32, kind="ExternalInput")
    z_d = nc.dram_tensor("zero1", [1, 1], F32, kind="ExternalInput")
    out_d = nc.dram_tensor("out", [T, K], F32, kind="ExternalOutput")

    with TileContext(nc) as tc:
        with (
            tc.tile_pool(name="sb", bufs=1) as sb,
            tc.tile_pool(name="wp", bufs=4) as wp,
            tc.tile_pool(name="ps", bufs=1, space="PSUM") as ps,
            tc.tile_pool(name="dram", bufs=1, space="DRAM") as dr,
        ):
            # ---- barrier AllReduce (4 bytes): aligns the 8 cores right at
            # kernel start so the big collectives later don't eat launch skew.
            zb = sb.tile([1, 1], F32)
            nc.gpsimd.memset(zb[:], 0.0)
            bzin = dr.tile([1, 1], F32)
            bzout = dr.tile([1, 1], F32, addr_space="Shared")
            nc.gpsimd.dma_start(bzin[:], zb[:])
            nc.gpsimd.collective_compute(
                "AllReduce",
                mybir.AluOpType.add,
                replica_groups=[list(range(NCORES))],
                ins=[bzin.opt()],
                outs=[bzout.opt()],
            )
            zsum = sb.tile([1, 1], F32)
            nc.scalar.dma_start(zsum[:], bzout[:])

            # X (784 KB) on the scalar HWDGE ring, W groups alternate between
            # the sync and scalar rings so both HWDGE FIFOs stream in parallel.
            # X lands in two pieces so the first matmul only waits for the
            # first GROUP's worth of X columns.
            # X's first 7 chunks (covering the first two W groups) go on the
            # scalar HWDGE ring; the rest of X and the tiny epilogue
            # constants ride the otherwise-idle gpsimd SWDGE ring so they
            # never delay a W group.
            xsb = sb.tile([128, NCH * T], F16)
            nc.scalar.dma_start(xsb[:, :7 * T], x_d[:, :7 * T])
            nc.gpsimd.dma_start(xsb[:, 7 * T:], x_d[:, 7 * T:])
            oc = sb.tile([T, 1], F32)
            nc.gpsimd.dma_start(oc[:], oc_d[:])
            o32 = sb.tile([T, T], BF16)
            nc.gpsimd.dma_start(o32[:], o32_d[:])
            orr = sb.tile([1, T], BF16)
            nc.gpsimd.dma_start(orr[:], or_d[:])
            tpos = sb.tile([T, 1], F32)
            nc.gpsimd.dma_start(tpos[:], tp_d[:])

            # ---- matmul: 98 accumulating X-stationary fp16 matmuls (N=512) --
            # The first W groups are small so the first matmul starts as soon
            # as possible; steady-state groups are 7 chunks (896 KB).
            accum = ps.tile([T, K], F32)
            base = 0
            for g, gsz in enumerate(GROUPS):
                wt = wp.tile([128, GROUP * K], F16, tag="wt")
                eng = nc.sync if (g % 2 == 0) else nc.scalar
                eng.dma_start(wt[:, :gsz * K],
                              w_d[:, base * K:(base + gsz) * K])
                for c in range(gsz):
                    cc = base + c
                    nc.tensor.matmul(
                        accum[:],
                        xsb[:, cc * T:(cc + 1) * T],
                        wt[:, c * K:(c + 1) * K],
                        start=(cc == 0),
                        stop=(cc == NCH - 1),
                    )
                base += gsz

            # ---- reduce the (32,512) fp32 partial across the 8 cores.
            # 3-stage pairwise-AllReduce butterfly (recursive doubling): the
            # runtime's own mesh/RS+AG plans at this size are step-latency
            # bound (~14 ncfw steps); the butterfly needs only 3 stages of
            # one-peer exchanges.
            part = sb.tile([T, K], F32)
            nc.vector.tensor_copy(part[:], accum[:])
            bin_ = dr.tile([T, K], F32)
            brs = dr.tile([T // NCORES, K], F32)
            nc.sync.dma_start(bin_[:], part[:])
            nc.gpsimd.collective_compute(
                "ReduceScatter",
                mybir.AluOpType.add,
                replica_groups=[list(range(NCORES))],
                ins=[bin_.opt()],
                outs=[brs.opt()],
            )
            bout = dr.tile([T, K], F32, addr_space="Shared")
            nc.gpsimd.collective_compute(
                "AllGather",
                mybir.AluOpType.bypass,
                replica_groups=[list(range(NCORES))],
                ins=[brs.opt()],
                outs=[bout.opt()],
            )
            ofull = sb.tile([T, K], F32)
            nc.sync.dma_start(ofull[:], bout[:])

            # ---- threshold fire ----
            # spike is 0.0/1.0 -> exact in bf16; lets the nspk matmul run at
            # 1 cycle/row.
            spike = sb.tile([T, K], BF16)
            nc.vector.tensor_scalar(spike[:], ofull[:], THRESH, None,
                                    op0=mybir.AluOpType.is_gt)
            pot = sb.tile([T, K], F32)
            nc.vector.scalar_tensor_tensor(pot[:], ofull[:], THRESH, ofull[:],
                                           op0=mybir.AluOpType.is_gt,
                                           op1=mybir.AluOpType.mult)

            # nspk broadcast to all 32 rows in one bf16 matmul (counts <= 32
            # are exact): ones(32,32).T @ spike
            nspkb_ps = ps.tile([T, K], F32)
            nc.tensor.matmul(nspkb_ps[:], o32[:], spike[:], start=True, stop=True)
            nrow = sb.tile([1, K], F32)
            nc.vector.tensor_copy(nrow[:], nspkb_ps[0:1, :])

            # onehot(t == clip(32-nspk,0,31)) == (nspk == 32 - t) except the
            # nspk==0 row-31 case, where pot is all-zero anyway.
            # values[k] = sum_t pot * onehot  (kept in fp32 for exactness)
            pv = sb.tile([T, K], F32)
            nc.vector.scalar_tensor_tensor(pv[:], nspkb_ps[:], tpos[:], pot[:],
                                           op0=mybir.AluOpType.is_equal,
                                           op1=mybir.AluOpType.mult)
            vals_ps = ps.tile([1, K], F32)
            nc.tensor.matmul(vals_ps[:], oc[:], pv[:], start=True, stop=True)

            # v = max(values * (nspk > 0)) * 32.  The (nspk > 0) mask is a
            # no-op: nspk == 0 implies pot (hence values) is all-zero, and
            # values >= 0 always, so reduce over values directly.
            vmax = sb.tile([1, 1], F32)
            nc.vector.tensor_reduce(vmax[:], vals_ps[:],
                                    axis=mybir.AxisListType.X,
                                    op=mybir.AluOpType.max)
            # vmax32 = vmax * 32 + 0.0 (the 0.0 comes from the barrier
            # AllReduce output, keeping it alive without numeric effect)
            vmax32 = sb.tile([1, 1], F32)
            nc.vector.scalar_tensor_tensor(vmax32[:], vmax[:], float(T), zsum[:],
                                           op0=mybir.AluOpType.mult,
                                           op1=mybir.AluOpType.add)

            # total = (values + vmax32) * nspk
            total = sb.tile([1, K], F32)
            nc.vector.scalar_tensor_tensor(total[:], vals_ps[:], vmax32[:],
                                           nrow[:],
                                           op0=mybir.AluOpType.add,
                                           op1=mybir.AluOpType.mult)

            # top-16 nonzero mask: two rounds of (8-max, match-replace-with-0).
            # Zero entries "win" as no-ops and never enter the mask, matching
            # the reference's invalid-winner (-1) behavior.  After the two
            # rounds, winners are exactly the entries `work` zeroed: features
            # with work == 0 and total == 0 never spike, so the mask
            # (work == 0) is equivalent to (total - work > 0) downstream.
            work = sb.tile([1, K], BF16)
            s8a = sb.tile([1, 8], F32)
            nc.vector.max(s8a[:], total[:])
            nc.vector.match_replace(work[:], s8a[:], total[:], 0.0)
            s8b = sb.tile([1, 8], BF16)
            nc.vector.max(s8b[:], work[:])
            nc.vector.match_replace(work[:], s8b[:], work[:], 0.0)

            # result = spike * (work_broadcast == 0)
            workb_ps = ps.tile([T, K], F32)
            nc.tensor.matmul(workb_ps[:], orr[:], work[:], start=True, stop=True)
            res = sb.tile([T, K], F32)
            nc.vector.scalar_tensor_tensor(res[:], workb_ps[:], 0.0, spike[:],
                                           op0=mybir.AluOpType.is_equal,
                                           op1=mybir.AluOpType.mult)
            nc.sync.dma_start(out_d[:], res[:])

    nc.compile()
    return nc


def _get_nc():
    if "nc" not in _CACHE:
        _CACHE["nc"] = _build_nc()
    return _CACHE["nc"]


def _pack_inputs(rec_field, weight):
    X = np.asarray(rec_field, dtype=np.float32).reshape(T, CTOT).astype(np.float16)
    W = np.asarray(weight, dtype=np.float32).reshape(K, CTOT).astype(np.float16)
    oc = np.ones((T, 1), np.float32)
    o32 = np.ones((T, T), np.float32).astype(mybir_np_bf16())
    orr = np.ones((1, T), np.float32).astype(mybir_np_bf16())
    tp = (float(T) - np.arange(T, dtype=np.float32)).reshape(T, 1)
    z1 = np.zeros((1, 1), np.float32)
    in_maps = []
    for i in range(NCORES):
        xp = np.zeros((T, SHP), np.float16)
        xp[:, :SH] = X[:, i * SH:(i + 1) * SH]
        wp = np.zeros((K, SHP), np.float16)
        wp[:, :SH] = W[:, i * SH:(i + 1) * SH]
        # (contract, n) -> chunks (NCH,128,n) -> partition-major (128, NCH*n)
        xpk = np.ascontiguousarray(
            xp.T.reshape(NCH, 128, T).transpose(1, 0, 2).reshape(128, NCH * T))
        wpk = np.ascontiguousarray(
            wp.T.reshape(NCH, 128, K).transpose(1, 0, 2).reshape(128, NCH * K))
        in_maps.append({"x": xpk, "w": wpk, "onescol": oc, "ones32": o32,
                        "onesrow": orr, "tpos32": tp, "zero1": z1})
    return in_maps


def mybir_np_bf16():
    import ml_dtypes
    return ml_dtypes.bfloat16


def kernel(rec_field, weight, _trace=False, _trace_kwargs=None):
    nc = _get_nc()
    in_maps = _pack_inputs(rec_field, weight)
    r = run_bass_kernel_spmd(nc, in_maps, list(range(NCORES)), trace=_trace,
                             **(_trace_kwargs or {}))
    _CACHE["last_results"] = r
    out = np.asarray(r.results[0]["out"], dtype=np.float32)
    return out.reshape(T, K, 1, 1)
